# revision 8
# baseline (speedup 1.0000x reference)
"""Trainium2 Bass kernel for RoPE linear attention (no softmax, strict causal).

Computes: QR = rope(Q); S = tril(QR @ QR^T, -1); out = S @ V
for Q [B=2, H=8, T=2048, N=1024], V [B,H,T,D=128], K == Q.

Sharding: B*H = 16 (b,h) pairs -> 2 per core across 8 cores (fully parallel).

Algorithm (chunked causal linear attention — O(T N D) instead of O(T^2 N)):
  out[t-blk i] = QR[i] @ C[g(i)] + tril_strict(QR[i] @ QR[j<=i in grp]^T) V
  where C[g] = sum_{s-blk < 2g} QR[s]^T V[s]  (prefix KV state, [N, D]),
  snapshotted at 2-block (256-row) group granularity; the within-group
  causal part is handled by masked score blocks (<=256 wide).

Engine assignment per (b,h):
  DMA:  Q tiles fp32 [128,1024], V (natural), tables (4MB, shared), out
  Pool: fp32->fp16 casts with pair de-interleave (evens->cols 0:512);
        the first 4 bh0 casts go to ACT (idle until the first qt drain)
  DVE:  rope in natural layout (6 tensor ops per 2-block span), masked
        score-strip drains, bh1's late qt/out drains (tail relief)
  PE:   transposes QN->QT, KV accumulation (PSUM-resident C, start=False
        onto memset zeros -- a start=True bracket per chunk would clear
        has_written bank-wide and clobber sibling regions), scores, QC, AV
  ACT:  QT drains, C snapshots (PSUM fp32 -> SBUF fp16), out drains

The two (b,h) pipelines are merged into one slot loop (bh1 offset by 5
slots) so bh1's fill/rope overlaps bh0's compute; out(g) is emitted one
slot late so the PE never waits on its own group's strip drain.
"""

import math
import os
import sys

import numpy as np

for _p in ("/opt/trn_rl_repo",):
    if _p not in sys.path and os.path.isdir(_p):
        sys.path.insert(0, _p)

THETA = 2 ** 16
B, H, T, N, D = 2, 8, 2048, 1024, 128
NB = T // 128          # 16 t-blocks
NG = NB // 2           # 8 groups of 2 blocks
NCH = N // 128         # 8 partition chunks
NPAIR = N // 2         # 512 rotation pairs
NC_COUNT = 8
BH_PER_CORE = (B * H) // NC_COUNT  # 2

_cache = {}


def _make_tables():
    """cos/sin tables in NATURAL pair-collapsed layout [128, NB*512] fp16.

    Row tp, col (b*512 + p) = cos/sin of phase(t = b*128 + tp, pair p).
    Phase arithmetic replicates reference._get_freqs/_rope in fp32 jnp ops
    on CPU, so the only table error is the final fp16 quantization.
    """
    import jax
    import jax.numpy as jnp

    with jax.default_device(jax.devices("cpu")[0]):
        pos = jnp.floor(jnp.arange(N, dtype=jnp.float32) / 2.0) * 2.0
        freqs = 1.0 / (THETA ** (pos / N)) / (2.0 * math.pi)        # (N,)
        r_phases = jnp.arange(T, dtype=jnp.float32)[:, None] * freqs[None, :]
        ph = (r_phases % 1.0) * (2.0 * math.pi)
        c = np.asarray(jnp.cos(ph))[:, 0::2]                         # (T, 512)
        s = np.asarray(jnp.sin(ph))[:, 0::2]
    cn = np.concatenate([c[b * 128:(b + 1) * 128] for b in range(NB)], axis=1)
    sn = np.concatenate([s[b * 128:(b + 1) * 128] for b in range(NB)], axis=1)
    return cn.astype(np.float16), sn.astype(np.float16)


def _build_nc(reps=1):
    import concourse.mybir as mybir
    from concourse import bacc
    from concourse.tile import TileContext

    f32 = mybir.dt.float32
    f16 = mybir.dt.float16

    cn_np, sn_np = _make_tables()
    tri = np.triu(np.ones((128, 128), np.float16), 1)   # keep s < t
    mask_np = np.concatenate([tri, np.ones((128, 128), np.float16), tri],
                             axis=1)                     # [128, 384]
    ident_np = np.eye(128, dtype=np.float16)

    nc = bacc.Bacc("TRN2", target_bir_lowering=False, debug=False,
                   num_devices=NC_COUNT)
    q = nc.dram_tensor("q", [BH_PER_CORE, T, N], f32, kind="ExternalInput")
    v = nc.dram_tensor("v", [BH_PER_CORE, T, D], f32, kind="ExternalInput")
    out = nc.dram_tensor("out", [BH_PER_CORE, T, D], f32,
                         kind="ExternalOutput")
    cn_dram = nc.inline_tensor(cn_np, name="cn_tab")
    sn_dram = nc.inline_tensor(sn_np, name="sn_tab")
    mask_dram = nc.inline_tensor(mask_np, name="mask_tab")
    ident_dram = nc.inline_tensor(ident_np, name="ident_tab")

    with TileContext(nc) as tc:
        with tc.tile_pool(name="const", bufs=1) as cpool, \
             tc.tile_pool(name="work", bufs=1) as pool, \
             tc.tile_pool(name="psT", bufs=2, space="PSUM") as psT, \
             tc.tile_pool(name="psC", bufs=1, space="PSUM") as psC, \
             tc.tile_pool(name="psS", bufs=2, space="PSUM") as psS, \
             tc.tile_pool(name="psO", bufs=1, space="PSUM") as psO:

            cn_sb = cpool.tile([128, NB * NPAIR], f16, name="cn")
            sn_sb = cpool.tile([128, NB * NPAIR], f16, name="sn")
            mask_sb = cpool.tile([128, 384], f16, name="mask")
            nc.sync.dma_start(out=mask_sb, in_=mask_dram[:, :])
            ident_sb = cpool.tile([128, 128], f16, name="ident")
            nc.sync.dma_start(out=ident_sb, in_=ident_dram[:, :])

            def load_tables(qtr):
                lo, hi = qtr * 4 * NPAIR, (qtr + 1) * 4 * NPAIR
                nc.sync.dma_start(out=cn_sb[:, lo:hi], in_=cn_dram[:, lo:hi])
                nc.sync.dma_start(out=sn_sb[:, lo:hi], in_=sn_dram[:, lo:hi])

            def one_rep(rp):
                P = f"r{rp}_"

                def make_bh(bh):
                    st = {}
                    st["qn"] = pool.tile([128, NB * N], f16, tag="qn", bufs=2,
                                         name=f"{P}qn{bh}")
                    st["qt"] = pool.tile([128, NCH * T], f16, tag="qt",
                                         bufs=2, name=f"{P}qt{bh}")
                    st["qt3"] = st["qt"].rearrange("p (c t) -> p c t", c=NCH)
                    st["vf"] = pool.tile([128, NB * D], f16, tag="vf", bufs=2,
                                         name=f"{P}vf{bh}")
                    return st

                def emit_vload(bh, st):
                    vs = pool.tile([128, NB * D], f32, tag="vstage", bufs=1,
                                   name=f"{P}vs{bh}")
                    nc.gpsimd.dma_start(
                        out=vs.rearrange("p (a d) -> p a d", a=NB),
                        in_=v[bh].rearrange("(a p) d -> p a d", p=128),
                    )
                    nc.gpsimd.tensor_copy(out=st["vf"], in_=vs)

                def emit_qload(bh, st, b, eng=None):
                    """Q tile DMA + de-interleave cast into qn."""
                    qs = pool.tile([128, N], f32, tag="qstage", bufs=3,
                                   name=f"{P}qs{bh}_{b}")
                    nc.sync.dma_start(out=qs,
                                      in_=q[bh, b * 128:(b + 1) * 128, :])
                    qn = st["qn"]
                    cp = (nc.scalar.copy if eng == "act"
                          else (lambda out, in_:
                                nc.gpsimd.tensor_copy(out=out, in_=in_)))
                    cp(qn[:, b * N: b * N + NPAIR], qs[:, 0:N:2])
                    cp(qn[:, b * N + NPAIR:(b + 1) * N], qs[:, 1:N:2])

                def emit_rope(bh, st, g):
                    """Rope group g (t-blocks 2g, 2g+1) in natural layout."""
                    qn3 = st["qn"].rearrange("p (b x) -> p b x", x=N)
                    bsl = slice(2 * g, 2 * g + 2)
                    qe = qn3[:, bsl, 0:NPAIR]
                    qo = qn3[:, bsl, NPAIR:N]
                    tb3 = cn_sb.rearrange("p (b x) -> p b x", x=NPAIR)
                    sb3 = sn_sb.rearrange("p (b x) -> p b x", x=NPAIR)
                    c_t = tb3[:, bsl, :]
                    s_t = sb3[:, bsl, :]
                    t1 = pool.tile([128, 2 * NPAIR], f16, tag="rt1", bufs=1,
                                   name=f"{P}t1_{bh}_{g}")
                    t2 = pool.tile([128, 2 * NPAIR], f16, tag="rt2", bufs=1,
                                   name=f"{P}t2_{bh}_{g}")
                    t13 = t1.rearrange("p (b x) -> p b x", x=NPAIR)
                    t23 = t2.rearrange("p (b x) -> p b x", x=NPAIR)
                    nc.vector.tensor_mul(out=t13, in0=qe, in1=s_t)
                    nc.vector.tensor_mul(out=t23, in0=qo, in1=s_t)
                    nc.vector.tensor_mul(out=qe, in0=qe, in1=c_t)
                    nc.vector.tensor_sub(out=qe, in0=qe, in1=t23)
                    nc.vector.tensor_mul(out=qo, in0=qo, in1=c_t)
                    nc.vector.tensor_add(out=qo, in0=qo, in1=t13)

                def emit_transpose(bh, st, b):
                    qn = st["qn"]
                    pt = psT.tile([128, N], f16, tag="pt",
                                  name=f"{P}pt{bh}_{b}")
                    for k in range(NCH):
                        nc.tensor.transpose(
                            pt[:, k * 128:(k + 1) * 128],
                            qn[:, b * N + k * 128: b * N + (k + 1) * 128],
                            ident_sb)
                    dst = st["qt3"][:, :, b * 128:(b + 1) * 128]
                    src3 = pt.rearrange("p (c t) -> p c t", c=NCH)
                    if bh == 1 and b >= 8:
                        # rope is done by the time these run; DVE is idle
                        # and its fp16-psum drain is faster than ACT's
                        nc.vector.tensor_copy(out=dst, in_=src3)
                    else:
                        nc.scalar.copy(dst, src3)

                def emit_kv(bh, st, cps, g):
                    """KV accumulation for s-blocks 2g, 2g+1 into C psum."""
                    qn = st["qn"]
                    for s in (2 * g, 2 * g + 1):
                        for k in range(NCH):
                            nc.tensor.matmul(
                                cps[:, k * 128:(k + 1) * 128],
                                lhsT=qn[:, s * N + k * 128:
                                        s * N + (k + 1) * 128],
                                rhs=st["vf"][:, s * D:(s + 1) * D],
                                start=False,
                                stop=(s == NB - 3),
                                skip_group_check=True,
                            )

                def emit_snap(bh, cps, g):
                    csb = pool.tile([128, N], f16, tag="csb", bufs=3,
                                    name=f"{P}csb{bh}_{g}")
                    nc.scalar.copy(csb, cps)
                    return csb

                def emit_scores(bh, st, g):
                    qt3 = st["qt3"]
                    sps = psS.tile([128, 384], f32, tag="sps",
                                   name=f"{P}sps{bh}_{g}")
                    c0 = 2 * g * 128
                    # brackets must be sequential per PSUM bank: start=True
                    # clears has_written bank-wide, so interleaving two
                    # accumulation groups in one bank clobbers the first
                    for k in range(NCH):
                        nc.tensor.matmul(
                            sps[:, 0:256],
                            lhsT=qt3[:, k, c0:c0 + 128],
                            rhs=qt3[:, k, c0:c0 + 256],
                            start=(k == 0), stop=(k == NCH - 1))
                    for k in range(NCH):
                        nc.tensor.matmul(
                            sps[:, 256:384],
                            lhsT=qt3[:, k, c0 + 128:c0 + 256],
                            rhs=qt3[:, k, c0 + 128:c0 + 256],
                            start=(k == 0), stop=(k == NCH - 1))
                    strip = pool.tile([128, 384], f16, tag="strip", bufs=2,
                                      name=f"{P}strip{bh}_{g}")
                    nc.vector.tensor_mul(out=strip, in0=sps, in1=mask_sb)
                    return strip

                def emit_out(bh, st, g, strip, csb_prev):
                    qt3 = st["qt3"]
                    vf = st["vf"]
                    ops2 = psO.tile([128, 2 * D], f32, tag="ops",
                                    name=f"{P}ops{bh}_{g}")
                    for ii, i in enumerate((2 * g, 2 * g + 1)):
                        ops = ops2[:, ii * D:(ii + 1) * D]
                        first = True
                        if csb_prev is not None:
                            for k in range(NCH):
                                nc.tensor.matmul(
                                    ops,
                                    lhsT=qt3[:, k, i * 128:(i + 1) * 128],
                                    rhs=csb_prev[:, k * 128:(k + 1) * 128],
                                    start=first, stop=False,
                                    skip_group_check=True)
                                first = False
                        if i == 2 * g:
                            nc.tensor.matmul(
                                ops, lhsT=strip[:, 0:128],
                                rhs=vf[:, i * D:(i + 1) * D],
                                start=first, stop=True,
                                skip_group_check=True)
                        else:
                            nc.tensor.matmul(
                                ops, lhsT=strip[:, 128:256],
                                rhs=vf[:, (i - 1) * D:i * D],
                                start=first, stop=False,
                                skip_group_check=True)
                            nc.tensor.matmul(
                                ops, lhsT=strip[:, 256:384],
                                rhs=vf[:, i * D:(i + 1) * D],
                                start=False, stop=True,
                                skip_group_check=True)
                    ob = pool.tile([128, 2 * D], f32, tag="ob", bufs=2,
                                   name=f"{P}ob{bh}_{g}")
                    if bh == 1 and g >= 4:
                        nc.vector.tensor_copy(out=ob, in_=ops2)
                    else:
                        nc.scalar.copy(ob, ops2)
                    nc.sync.dma_start(
                        out=out[bh, 2 * g * 128:(2 * g + 2) * 128, :]
                            .rearrange("(i p) d -> p i d", p=128),
                        in_=ob.rearrange("p (i d) -> p i d", i=2))

                # ---- schedule: merged-slot software pipeline ---------
                st0 = make_bh(0)
                st1 = make_bh(1)

                for b in range(NB):
                    # ACT is idle until the first qt drains (~13us): give it
                    # the early casts so Pool doesn't gate the rope
                    emit_qload(0, st0, b, eng="act" if b < 4 else None)
                    if b == 2:
                        emit_vload(0, st0)
                    if rp == 0 and b % 4 == 1:
                        load_tables(b // 4)
                emit_rope(0, st0, 0)
                emit_rope(0, st0, 1)

                # C accumulates via start=False matmuls onto memset zeros:
                # a start=True bracket per chunk region would clear
                # has_written for the whole bank and clobber sibling regions
                cps = [psC.tile([128, N], f32, tag="cps", name=f"{P}cps{x}")
                       for x in range(2)]
                nc.scalar.memzero(cps[0])

                sts = [st0, st1]
                strips = [{}, {}]
                csbs = [{}, {}]

                def slot(bh, m, part):
                    """part 0: T+KV+snap; 1: S; 2: out(m-1)."""
                    st = sts[bh]
                    if part == 0:
                        emit_transpose(bh, st, 2 * m)
                        if m < NG - 1:
                            emit_kv(bh, st, cps[bh], m)
                        emit_transpose(bh, st, 2 * m + 1)
                        if m < NG - 1:
                            csbs[bh][m] = emit_snap(bh, cps[bh], m)
                    elif part == 1:
                        strips[bh][m] = emit_scores(bh, st, m)
                    else:
                        if m >= 1:
                            emit_out(bh, st, m - 1, strips[bh][m - 1],
                                     csbs[bh].get(m - 2))

                OFF = 5
                for m in range(NG + OFF):
                    j = m - OFF
                    if m < NG:
                        slot(0, m, 0)
                        slot(0, m, 1)
                    if 0 <= j < NG:
                        slot(1, j, 0)
                    if m <= NG:
                        slot(0, m, 2)
                    if 0 <= j < NG:
                        slot(1, j, 1)
                        slot(1, j, 2)
                    # DVE/DMA/Pool interleaves at bh0-slot cadence
                    if m == 0:
                        emit_vload(1, st1)
                    if m + 2 < NG:
                        emit_rope(0, st0, m + 2)
                    if m < NG:
                        emit_qload(1, st1, 2 * m)
                        emit_qload(1, st1, 2 * m + 1)
                    if m == 7:
                        nc.scalar.memzero(cps[1])
                    if 4 <= m <= NG + 3:
                        emit_rope(1, st1, m - 4)
                emit_out(1, st1, NG - 1, strips[1][NG - 1], csbs[1][NG - 2])

            for rp in range(reps):
                one_rep(rp)

    nc.compile()
    return nc


def _get_nc():
    if "nc" not in _cache:
        _cache["nc"] = _build_nc()
    return _cache["nc"]


def kernel(Q, K, V):
    from concourse import bass_utils

    del K  # K is Q by construction
    Qr = np.ascontiguousarray(Q.reshape(B * H, T, N), dtype=np.float32)
    Vr = np.ascontiguousarray(V.reshape(B * H, T, D), dtype=np.float32)

    nc = _get_nc()
    in_maps = []
    for c in range(NC_COUNT):
        lo = c * BH_PER_CORE
        in_maps.append({
            "q": np.ascontiguousarray(Qr[lo:lo + BH_PER_CORE]),
            "v": np.ascontiguousarray(Vr[lo:lo + BH_PER_CORE]),
        })

    res = bass_utils.run_bass_kernel_spmd(
        nc, in_maps, core_ids=list(range(NC_COUNT)),
    )
    _cache["last_result"] = res
    outs = [res.results[c]["out"].reshape(BH_PER_CORE, T, D)
            for c in range(NC_COUNT)]
    return np.concatenate(outs, axis=0).reshape(B, H, T, D).astype(np.float32)
